# revision 2
# baseline (speedup 1.0000x reference)
# Trainium2 Bass kernel for nn_ConPrediction (gnn_message_passing).
#
# Strategy (8-way SPMD, one program on cores 0-7):
#   * GNN: row-shard adjacency. Each core holds A[rows_c,:]^T (host
#     pre-transposed) in SBUF; per layer computes its hs slice, AllGathers
#     hs in a block-transposed [80, 625] layout, PE-transposes chunks back
#     to natural layout for use as the stationary matmul operand, and
#     accumulates A_local @ hs into its xs slice.  All data is kept in a
#     transposed [dim, rows] layout so row-shifts for the 11x11 convs
#     become free-dim offset slices; biases are applied via the scalar
#     engine's per-partition bias argument (out partitions = feature dim).
#   * Convs: out = sum_dr M_dr^T @ img_T[:, shifted], M_dr the 10x10
#     width-conv band matrix for row-offset dr (host-built from Wc_k).
#     Row-sharded with a 15-row halo; global-edge cols re-zeroed with tiny
#     per-core edge masks after each layer.
#   * Attention means: w = tanh(q^T hs) via matmul, partition_broadcast of
#     w, then one tensor_tensor_reduce -> per-dim partial sums; AllReduce.
#   * All mean scale factors folded into host-prepared weights.
import os
import numpy as np

NC = 8
DIM = 10
WIN = 5
L_GNN = L_CNN = L_NN = L_OUT = 3
N_ATOMS = 5000
N_SEQ = 8000
L_PSSM = 2000
N_FP = 100000
N_VOCAB = 10000

RA = N_ATOMS // NC   # 625 atom rows per core
RW = N_SEQ // NC     # 1000 word rows per core
RP = L_PSSM // NC    # 250 pssm rows per core
HALO = 3 * WIN       # 15

FP_CH = 5    # ceil(625/128)
W_CH = 9     # ceil(1030/128)
C_CH = 6     # ceil(655/128)

# small10 pack layout: 15 blocks of 10 cols each
B_GNN = 0
B_WN = 3
B_PO = 6
B_WA = 9
B_WA_RD = 10
B_WA_PS = 11
B_WP_SUB = 12
B_WP_PRO = 13
B_WP_PLAIN = 14

# b10 bias pack cols
C_GNN = 0   # 3 cols
C_WN = 3    # 3 cols
C_PO = 6    # 3 cols
C_WA = 9    # 1 col
C_CONV = 10  # 3 cols
C_WP = 13   # 1 col

_CACHE = {}


def _chunks(total, step=512):
    return [(s, min(step, total - s)) for s in range(0, total, step)]


def _chunks2(total, step=512):
    # balanced EVEN chunks <= step (fp32r needs an even moving dim and
    # runs at full rate only when chunks are >= 256)
    import math
    n = max(1, math.ceil(total / step))
    out, s, rem = [], 0, total
    for i in range(n):
        sz = rem // (n - i)
        if i < n - 1:
            sz = (sz + 1) // 2 * 2
        out.append((s, sz))
        s += sz
        rem -= sz
    return out


def _build_program(stub_collectives=False):
    import concourse.bacc as bacc
    import concourse.bass as bass
    import concourse.tile as tile
    from concourse import mybir
    from concourse.masks import make_identity
    from contextlib import ExitStack

    f32 = mybir.dt.float32
    i32 = mybir.dt.int32
    AF = mybir.ActivationFunctionType
    ALU = mybir.AluOpType
    RG = [list(range(NC))]

    nc = bacc.Bacc(
        "TRN2", target_bir_lowering=False, debug=False, num_devices=NC
    )

    # ---------- I/O ----------
    f32r = mybir.dt.float32r
    bf16 = mybir.dt.bfloat16
    at_dram = nc.dram_tensor("at_shard", [N_ATOMS, RA + 1], bf16, kind="ExternalInput")
    embfp_d = nc.dram_tensor("emb_fp", [N_FP, DIM], f32, kind="ExternalInput")
    embw_d = nc.dram_tensor("emb_word", [N_VOCAB, DIM], f32, kind="ExternalInput")
    fpidx_d = nc.dram_tensor("fp_idx", [128, FP_CH], i32, kind="ExternalInput")
    widx_d = nc.dram_tensor("word_idx", [128, W_CH], i32, kind="ExternalInput")
    selL_d = nc.dram_tensor("selL", [NC * DIM, DIM], f32r, kind="ExternalInput")
    selR_d = nc.dram_tensor("selR", [NC * DIM, DIM], f32r, kind="ExternalInput")
    pssm_d = nc.dram_tensor("pssm_b0", [DIM, RP + 36], f32r, kind="ExternalInput")
    rdkit_d = nc.dram_tensor("rdkit_t", [DIM, 200], f32, kind="ExternalInput")
    maskw_d = nc.dram_tensor("mask_w", [DIM, RW + 30], f32, kind="ExternalInput")
    maskc_d = nc.dram_tensor("mask_c", [DIM, RA + 30], f32, kind="ExternalInput")
    maskp_d = nc.dram_tensor("mask_p", [DIM, RP + 30], f32, kind="ExternalInput")
    convw_d = nc.dram_tensor("convw", [DIM, 330], f32r, kind="ExternalInput")
    s10_d = nc.dram_tensor("small10", [DIM, 150], f32r, kind="ExternalInput")
    b10_d = nc.dram_tensor("b10", [DIM, 14], f32, kind="ExternalInput")
    w30_d = nc.dram_tensor("w30", [3 * DIM, 91], f32, kind="ExternalInput")
    wob_d = nc.dram_tensor("wob", [3 * DIM, 3], f32, kind="ExternalInput")
    wopair_d = nc.dram_tensor("wo_pair", [DIM, 3 * DIM], f32, kind="ExternalInput")
    wpbrow_d = nc.dram_tensor("wpb_row", [1, DIM], f32, kind="ExternalInput")
    wib_d = nc.dram_tensor("wib", [1, 1], f32, kind="ExternalInput")
    ones_d = nc.dram_tensor("ones10", [1, DIM], f32r, kind="ExternalInput")
    zeros_d = nc.dram_tensor("zeros10", [1, DIM], f32r, kind="ExternalInput")
    out_d = nc.dram_tensor("out", [1, 1], f32, kind="ExternalOutput")

    with tile.TileContext(nc) as tc, ExitStack() as ctx:
        sb = ctx.enter_context(tc.tile_pool(name="sb", bufs=1))
        sb2 = ctx.enter_context(tc.tile_pool(name="sb2", bufs=2))
        st3 = ctx.enter_context(tc.tile_pool(name="st3", bufs=3))
        ps_acc = ctx.enter_context(tc.tile_pool(name="ps_acc", bufs=1, space="PSUM"))
        ps_mm = ctx.enter_context(tc.tile_pool(name="ps_mm", bufs=3, space="PSUM"))
        ps_tr = ctx.enter_context(tc.tile_pool(name="ps_tr", bufs=2, space="PSUM"))
        dr = ctx.enter_context(tc.tile_pool(name="dr", bufs=1, space="DRAM"))

        _scope_stack = []

        def scope(name):
            sid, _ = nc.enter_named_scope(name, False)
            _scope_stack.append((name, sid))

        def escope():
            name, sid = _scope_stack.pop()
            nc.leave_named_scope(name, sid, False)

        # ---------- constants / weights into SBUF ----------
        ident = sb.tile([128, 128], f32, name="ident")
        make_identity(nc, ident[:, :])

        convw = sb.tile([DIM, 330], f32r, name="convw_sb")
        nc.sync.dma_start(out=convw[:, :], in_=convw_d[:, :])
        s10 = sb.tile([DIM, 150], f32r, name="s10_sb")
        nc.sync.dma_start(out=s10[:, :], in_=s10_d[:, :])
        b10 = sb.tile([DIM, 14], f32, name="b10_sb")
        nc.sync.dma_start(out=b10[:, :], in_=b10_d[:, :])
        w30 = sb.tile([3 * DIM, 91], f32, name="w30_sb")
        nc.sync.dma_start(out=w30[:, :], in_=w30_d[:, :])
        wob = sb.tile([3 * DIM, 3], f32, name="wob_sb")
        nc.sync.dma_start(out=wob[:, :], in_=wob_d[:, :])
        wopair = sb.tile([DIM, 3 * DIM], f32, name="wopair_sb")
        nc.sync.dma_start(out=wopair[:, :], in_=wopair_d[:, :])
        wpbrow = sb.tile([1, DIM], f32, name="wpbrow_sb")
        nc.sync.dma_start(out=wpbrow[:, :], in_=wpbrow_d[:, :])
        wib = sb.tile([1, 1], f32, name="wib_sb")
        nc.sync.dma_start(out=wib[:, :], in_=wib_d[:, :])

        def s10w(block):  # [10, 10] lhsT slice
            return s10[0:DIM, block * 10 : block * 10 + 10]

        def bcol(col):  # [10, 1] bias slice
            return b10[0:DIM, col : col + 1]

        def convw_l(l, dri):  # [10, 10] lhsT slice
            o = (l * 11 + dri) * 10
            return convw[0:DIM, o : o + 10]

        def r32(ap):
            return ap.bitcast(f32r)

        def c32(ap):
            return ap.bitcast(f32)

        maskw = sb.tile([DIM, RW + 30], f32, name="maskw_sb")
        nc.sync.dma_start(out=maskw[:, :], in_=maskw_d[:, :])
        maskc = sb.tile([DIM, RA + 30], f32, name="maskc_sb")
        nc.sync.dma_start(out=maskc[:, :], in_=maskc_d[:, :])
        maskp = sb.tile([DIM, RP + 30], f32, name="maskp_sb")
        nc.sync.dma_start(out=maskp[:, :], in_=maskp_d[:, :])

        fpidx = sb.tile([128, FP_CH], i32, name="fpidx_sb")
        nc.sync.dma_start(out=fpidx[:, :], in_=fpidx_d[:, :])
        widx = sb.tile([128, W_CH], i32, name="widx_sb")
        nc.sync.dma_start(out=widx[:, :], in_=widx_d[:, :])
        selL = sb.tile([NC * DIM, DIM], f32r, name="selL_sb")
        nc.sync.dma_start(out=selL[:, :], in_=selL_d[:, :])
        selR = sb.tile([NC * DIM, DIM], f32r, name="selR_sb")
        nc.sync.dma_start(out=selR[:, :], in_=selR_d[:, :])

        # ---------- A^T tiles (12.5 MB, streamed while the rest runs) ----
        at_t = {}
        scope("atdma")
        for cc in range(NC):
            for m in range(5):
                sz = min(128, RA - m * 128)
                t = sb.tile([128, RA + 1], bf16, name=f"at_{cc}_{m}", tag=f"at_{cc}_{m}")
                r0 = cc * RA + m * 128
                nc.sync.dma_start(out=t[0:sz, :], in_=at_dram[r0 : r0 + sz, :])
                at_t[(cc, m)] = t
        escope()

        # ---------- helpers ----------
        def gather_transpose_into(dstbuf, dst_cols, table_ap, idx_tile, nch, tagp):
            for m in range(nch):
                n = min(128, dst_cols - m * 128)
                if n <= 0:
                    break
                st = st3.tile([128, DIM], f32, name=f"{tagp}_st{m}", tag=f"{tagp}_st")
                nc.gpsimd.indirect_dma_start(
                    out=st[:, :],
                    out_offset=None,
                    in_=table_ap,
                    in_offset=bass.IndirectOffsetOnAxis(
                        ap=idx_tile[:, m : m + 1], axis=0
                    ),
                )
                tp = ps_tr.tile([DIM, 128], f32, name=f"{tagp}_tp{m}", tag="tr")
                nc.tensor.transpose(
                    out=tp[0:DIM, 0:128], in_=st[0:128, 0:DIM],
                    identity=ident[0:128, 0:128],
                )
                nc.vector.tensor_copy(
                    dstbuf[0:DIM, m * 128 : m * 128 + n], tp[0:DIM, 0:n]
                )

        def apply_edge_masks(buf, span, mask_sb, moff):
            nc.vector.tensor_mul(
                buf[0:DIM, 0:16], buf[0:DIM, 0:16], mask_sb[0:DIM, moff : moff + 16]
            )
            nc.vector.tensor_mul(
                buf[0:DIM, span - 16 : span],
                buf[0:DIM, span - 16 : span],
                mask_sb[0:DIM, moff + span - 16 : moff + span],
            )

        def conv_layer(bufin, l, span_mm, span_valid, tagp, mask_sb, moff,
                       mask_out):
            """One conv layer: bufin [10, span_mm+10] -> [10, span_mm].
            span_mm is even (fp32r); cols >= span_valid are finite junk that
            no valid downstream column ever reads."""
            nbuf = sb.tile([DIM, span_mm], f32r, name=f"{tagp}_b{l}",
                           tag=f"{tagp}_b{l}")
            for (cs, n) in _chunks2(span_mm):
                pscv = ps_mm.tile([DIM, 512], f32, name=f"{tagp}_ps{l}", tag="mm")
                for dri in range(11):
                    nc.tensor.matmul(
                        out=pscv[0:DIM, 0:n],
                        lhsT=convw_l(l, dri),
                        rhs=bufin[0:DIM, dri + cs : dri + cs + n],
                        start=(dri == 0),
                        stop=(dri == 10),
                    )
                nc.scalar.activation(
                    out=nbuf[0:DIM, cs : cs + n], in_=pscv[0:DIM, 0:n],
                    func=AF.Relu, bias=bcol(C_CONV + l),
                )
            if mask_out:
                apply_edge_masks(nbuf, span_valid, mask_sb, moff)
            return nbuf

        def hs_from(buf, L, out_name, tagp):
            """relu(wa.T @ buf + wa_b) for buf [10, L] -> [10, L] SBUF."""
            hst = sb.tile([DIM, L], f32r, name=out_name, tag=out_name)
            for (cs, n) in _chunks2(L):
                p = ps_mm.tile([DIM, 512], f32, name=f"{tagp}_hsps", tag="mm")
                nc.tensor.matmul(
                    out=p[0:DIM, 0:n], lhsT=s10w(B_WA),
                    rhs=buf[0:DIM, cs : cs + n],
                )
                nc.scalar.activation(
                    out=hst[0:DIM, cs : cs + n], in_=p[0:DIM, 0:n],
                    func=AF.Relu, bias=bcol(C_WA),
                )
            return hst

        ones_row = sb.tile([1, DIM], f32r, name="ones_row")
        nc.sync.dma_start(out=ones_row[:, :], in_=ones_d[:, :])

        def attention_tail(hs_t, L, valid, q_tile, out_part, tagp):
            """w = tanh(q^T hs); S = sum_r w[r]*hs[:,r] -> out_part [10,1].
            The [1,L] w row is broadcast across partitions with a rank-1
            matmul (ones_row^T @ w), then mul + free-axis reduce."""
            w_c = sb.tile([1, L], f32r, name=f"{tagp}_wc", tag=f"{tagp}_wc")
            cl = _chunks2(L)
            for (cs, n) in cl:
                p = ps_mm.tile([1, 512], f32, name=f"{tagp}_wps", tag="mm")
                nc.tensor.matmul(
                    out=p[0:1, 0:n], lhsT=q_tile[0:DIM, 0:1],
                    rhs=hs_t[0:DIM, cs : cs + n],
                )
                nc.scalar.activation(
                    out=w_c[0:1, cs : cs + n], in_=p[0:1, 0:n], func=AF.Tanh
                )
            S_tmp = sb.tile([DIM, len(cl)], f32, name=f"{tagp}_stmp",
                            tag=f"{tagp}_stmp")
            for ci, (cs, n) in enumerate(cl):
                nv = min(cs + n, valid) - cs
                pb = ps_mm.tile([DIM, 512], f32, name=f"{tagp}_bc", tag="mm")
                nc.tensor.matmul(
                    out=pb[0:DIM, 0:n], lhsT=ones_row[0:1, 0:DIM],
                    rhs=w_c[0:1, cs : cs + n],
                )
                scr = st3.tile([DIM, 512], f32, name=f"{tagp}_scr", tag="att_scr")
                nc.vector.tensor_mul(
                    scr[0:DIM, 0:nv], hs_t[0:DIM, cs : cs + nv], pb[0:DIM, 0:nv]
                )
                nc.vector.reduce_sum(
                    out=S_tmp[0:DIM, ci : ci + 1], in_=scr[0:DIM, 0:nv],
                    axis=mybir.AxisListType.X,
                )
            nc.vector.reduce_sum(
                out=out_part[0:DIM, 0:1], in_=S_tmp[0:DIM, 0 : len(cl)],
                axis=mybir.AxisListType.X,
            )

        # =========== stage 1: fingerprint gather -> xsT ==================
        scope("fp_gather")
        xsT = sb.tile([DIM, RA + 1], f32r, name="xsT")
        gather_transpose_into(xsT, RA, embfp_d[:, :], fpidx, FP_CH, "fpg")
        nc.sync.dma_start(out=xsT[0:DIM, RA : RA + 1], in_=zeros_d[0:1, 0:DIM])
        escope()

        # =========== words buf0 (gather + transpose + mask) ==============
        scope("w_gather")
        wspan0 = RW + 30
        wbuf0 = sb.tile([DIM, wspan0], f32r, name="wbuf0")
        gather_transpose_into(wbuf0, wspan0, embw_d[:, :], widx, W_CH, "wg")
        apply_edge_masks(wbuf0, wspan0, maskw, 0)
        escope()

        # =========== GNN layer helper ====================================
        def gnn_hs_and_ag(l):
            scope(f"gnn_hs{l}")
            hsT = sb2.tile([DIM, RA + 1], f32, name=f"hsT_{l}", tag="hsT")
            for (cs, n) in _chunks2(RA + 1):
                ph = ps_mm.tile([DIM, 512], f32, name=f"ph_{l}", tag="mm")
                nc.tensor.matmul(
                    out=ph[0:DIM, 0:n], lhsT=s10w(B_GNN + l),
                    rhs=xsT[0:DIM, cs : cs + n],
                )
                nc.scalar.activation(
                    out=hsT[0:DIM, cs : cs + n], in_=ph[0:DIM, 0:n],
                    func=AF.Relu, bias=bcol(C_GNN + l),
                )
            ag_in = dr.tile([DIM, RA], f32, name=f"hs_agin_{l}")
            ag_out = dr.tile([NC * DIM, RA], f32, name=f"hs_agout_{l}",
                             addr_space="Local" if stub_collectives else "Shared")
            nc.sync.dma_start(out=ag_in[:, :], in_=hsT[0:DIM, 0:RA])
            if stub_collectives:
                for c in range(NC):
                    nc.sync.dma_start(
                        out=ag_out[c * DIM : (c + 1) * DIM, :], in_=ag_in[:, :]
                    )
            else:
                nc.gpsimd.collective_compute(
                    "AllGather", ALU.bypass, replica_groups=RG,
                    ins=[ag_in[:, :].opt()], outs=[ag_out[:, :].opt()],
                )
            escope()
            return ag_out

        def gnn_aggregate(l, ag_out, halo_cb=None):
            scope(f"gnn_agg{l}")
            hall = sb2.tile([NC * DIM, RA], f32, name=f"hall_{l}", tag="hall")
            nc.sync.dma_start(out=hall[:, :], in_=ag_out[:, :])
            hs_nat = []
            for m in range(5):
                sz = min(128, RA - m * 128)
                tp = ps_tr.tile([128, NC * DIM], f32, name=f"htr_{l}_{m}", tag="tr")
                nc.tensor.transpose(
                    out=tp[0:sz, 0 : NC * DIM],
                    in_=hall[0 : NC * DIM, m * 128 : m * 128 + sz],
                    identity=ident[0 : NC * DIM, 0 : NC * DIM],
                )
                hn = sb2.tile([128, NC * DIM], bf16, name=f"hn_{l}_{m}", tag=f"hn{m}")
                nc.vector.tensor_copy(hn[0:sz, 0 : NC * DIM], tp[0:sz, 0 : NC * DIM])
                hs_nat.append(hn)
            if halo_cb is not None:
                halo_cb(hs_nat)
            nch = _chunks2(RA + 1)
            pgs = [
                ps_acc.tile([DIM, 320], f32, name=f"pg{i}_{l}", tag=f"pg{i}")
                for i in range(len(nch))
            ]
            nblk = NC * 5
            k = 0
            for m in range(5):
                sz = min(128, RA - m * 128)
                for cc in range(NC):
                    lhsT = hs_nat[m][0:sz, cc * DIM : (cc + 1) * DIM]
                    rhs = at_t[(cc, m)]
                    for i, (cs, n) in enumerate(nch):
                        nc.tensor.matmul(
                            out=pgs[i][0:DIM, 0:n], lhsT=lhsT,
                            rhs=rhs[0:sz, cs : cs + n],
                            start=(k == 0), stop=(k == nblk - 1),
                        )
                    k += 1
            for i, (cs, n) in enumerate(nch):
                nv = min(cs + n, RA) - cs
                nc.vector.tensor_add(
                    xsT[0:DIM, cs : cs + nv], xsT[0:DIM, cs : cs + nv],
                    pgs[i][0:DIM, 0:nv]
                )
            escope()

        # =========== GNN layer 0 hs + AG (launch early) ==================
        ag0 = gnn_hs_and_ag(0)

        # =========== rdkit chain (fills AG0 wait) ========================
        scope("rdkit")
        rdx = sb.tile([DIM, 200], f32, name="rdx0")
        nc.sync.dma_start(out=rdx[:, :], in_=rdkit_d[:, :])
        for l in range(L_NN):
            p = ps_mm.tile([DIM, 512], f32, name=f"rd_ps{l}", tag="mm")
            nc.tensor.matmul(
                out=p[0:DIM, 0:200], lhsT=c32(s10w(B_WN + l)), rhs=rdx[0:DIM, 0:200]
            )
            rdn = sb2.tile([DIM, 200], f32, name=f"rdx{l + 1}", tag="rdx")
            nc.scalar.activation(
                out=rdn[0:DIM, 0:200], in_=p[0:DIM, 0:200],
                func=AF.Relu, bias=bcol(C_WN + l),
            )
            rdx = rdn
        # h = relu(wa @ x + b), fused row-sum -> S_rd
        S_rd = sb.tile([DIM, 1], f32, name="S_rd")
        prd = ps_mm.tile([DIM, 512], f32, name="rd_hps", tag="mm")
        nc.tensor.matmul(
            out=prd[0:DIM, 0:200], lhsT=c32(s10w(B_WA)), rhs=rdx[0:DIM, 0:200]
        )
        rd_scr = sb.tile([DIM, 200], f32, name="rd_scr")
        nc.scalar.activation(
            out=rd_scr[0:DIM, 0:200], in_=prd[0:DIM, 0:200], func=AF.Relu,
            bias=bcol(C_WA),
        )
        nc.vector.reduce_sum(
            out=S_rd[0:DIM, 0:1], in_=rd_scr[0:DIM, 0:200],
            axis=mybir.AxisListType.X,
        )
        # substrate attention query h_q = relu(wa_rd.T @ S_rd + wa_b)
        h_q = sb.tile([DIM, 1], f32r, name="h_q")
        pq = ps_mm.tile([DIM, 512], f32, name="hq_ps", tag="mm")
        nc.tensor.matmul(
            out=pq[0:DIM, 0:1], lhsT=c32(s10w(B_WA_RD)), rhs=S_rd[0:DIM, 0:1]
        )
        nc.scalar.activation(
            out=h_q[0:DIM, 0:1], in_=pq[0:DIM, 0:1], func=AF.Relu, bias=bcol(C_WA)
        )
        escope()

        # =========== pssm conv + partial sum + its own AllReduce =========
        scope("pssm")
        pbuf = sb.tile([DIM, RP + 36], f32r, name="pbuf0")
        nc.sync.dma_start(out=pbuf[:, :], in_=pssm_d[:, :])
        pb = pbuf
        for l in range(L_CNN):
            span = RP + 30 - 10 * (l + 1)
            pb = conv_layer(pb, l, span + 6, span, "pcv", maskp, 5 * (l + 1),
                            l < L_CNN - 1)
        hp_ps = ps_mm.tile([DIM, 512], f32, name="hp_ps", tag="mm")
        nc.tensor.matmul(
            out=hp_ps[0:DIM, 0:RP], lhsT=c32(s10w(B_WA)), rhs=c32(pb[0:DIM, 0:RP])
        )
        S_ps_part = sb.tile([DIM, 1], f32, name="S_ps_part")
        pp_scr = sb.tile([DIM, RP], f32, name="pp_scr")
        nc.scalar.activation(
            out=pp_scr[0:DIM, 0:RP], in_=hp_ps[0:DIM, 0:RP], func=AF.Relu,
            bias=bcol(C_WA),
        )
        nc.vector.reduce_sum(
            out=S_ps_part[0:DIM, 0:1], in_=pp_scr[0:DIM, 0:RP],
            axis=mybir.AxisListType.X,
        )
        arp_in = dr.tile([1, DIM], f32, name="arp_in")
        arp_out = dr.tile([1, DIM], f32, name="arp_out",
                          addr_space="Local" if stub_collectives else "Shared")
        nc.sync.dma_start(out=arp_in[0:1, 0:DIM], in_=S_ps_part[0:DIM, 0:1])
        if stub_collectives:
            nc.sync.dma_start(out=arp_out[:, :], in_=arp_in[:, :])
        else:
            nc.gpsimd.collective_compute(
                "AllReduce", ALU.add, replica_groups=RG,
                ins=[arp_in[:, :].opt()], outs=[arp_out[:, :].opt()],
            )
        S_ps_tot = sb.tile([DIM, 1], f32, name="S_ps_tot")
        nc.sync.dma_start(out=S_ps_tot[0:DIM, 0:1], in_=arp_out[0:1, 0:DIM])
        h_qp = sb.tile([DIM, 1], f32r, name="h_qp")
        pq2 = ps_mm.tile([DIM, 512], f32, name="hqp_ps", tag="mm")
        nc.tensor.matmul(
            out=pq2[0:DIM, 0:1], lhsT=c32(s10w(B_WA_PS)), rhs=S_ps_tot[0:DIM, 0:1]
        )
        nc.scalar.activation(
            out=h_qp[0:DIM, 0:1], in_=pq2[0:DIM, 0:1], func=AF.Relu, bias=bcol(C_WA)
        )
        escope()

        # =========== words conv layer 1 (fills AG0/AT-DMA wait) ==========
        scope("wconv0")
        wb = wbuf0
        wb = conv_layer(wb, 0, RW + 20, RW + 20, "wcv", maskw, 5, True)
        escope()

        # =========== GNN layer 0 aggregate + layer 1 hs/AG ===============
        gnn_aggregate(0, ag0)
        ag1 = gnn_hs_and_ag(1)

        # =========== words conv layer 2 ==================================
        scope("wconv1")
        wb = conv_layer(wb, 1, RW + 10, RW + 10, "wcv", maskw, 10, True)
        escope()

        # =========== GNN layer 1 aggregate + layer 2 hs/AG ===============
        gnn_aggregate(1, ag1)
        ag2 = gnn_hs_and_ag(2)

        # =========== words conv layer 3 + hs_w ===========================
        scope("wconv2")
        wb = conv_layer(wb, 2, RW, RW, "wcv", maskw, 15, False)
        hs_w = hs_from(wb, RW, "hs_w", "wat")
        escope()

        # =========== GNN layer 2 aggregate -> compound; natural AG =======
        # layer-2 aggregate computes the 32 halo columns of A@hs with tiny
        # edge matmuls FIRST, so the compound-halo AllGather overlaps the
        # main 625-wide matmul block.
        cg_in = dr.tile([DIM, 32], f32, name="cg_in")
        cg_out = dr.tile([NC * DIM, 32], f32, name="cg_out",
                         addr_space="Local" if stub_collectives else "Shared")

        def halo_early(hs_nat):
            scope("cg_ag")
            pLR = ps_acc.tile([DIM, 32], f32, name="pLR", tag="trh")
            nblk = NC * 5
            k = 0
            for m in range(5):
                sz = min(128, RA - m * 128)
                for cc in range(NC):
                    lhsT = hs_nat[m][0:sz, cc * DIM : (cc + 1) * DIM]
                    s = at_t[(cc, m)][0:sz, 0:626]
                    # one moving stream over cols [0:16] + [609:625]
                    rhs = bass.AP(s.tensor, s.offset, [s.ap[0], [609, 2], [1, 16]])
                    nc.tensor.matmul(
                        out=pLR[0:DIM, 0:32], lhsT=lhsT, rhs=rhs,
                        start=(k == 0), stop=(k == nblk - 1),
                    )
                    k += 1
            hpk = sb.tile([DIM, 32], f32r, name="hpk")
            nc.vector.tensor_add(hpk[0:DIM, 0:16], xsT[0:DIM, 0:16],
                                 pLR[0:DIM, 0:16])
            nc.vector.tensor_add(hpk[0:DIM, 16:32], xsT[0:DIM, 609:625],
                                 pLR[0:DIM, 16:32])
            nc.sync.dma_start(out=cg_in[:, :], in_=c32(hpk[0:DIM, 0:32]))
            if stub_collectives:
                for c in range(NC):
                    nc.sync.dma_start(
                        out=cg_out[c * DIM : (c + 1) * DIM, :], in_=cg_in[:, :]
                    )
            else:
                nc.gpsimd.collective_compute(
                    "AllGather", ALU.bypass, replica_groups=RG,
                    ins=[cg_in[:, :].opt()], outs=[cg_out[:, :].opt()],
                )
            escope()

        gnn_aggregate(2, ag2, halo_cb=halo_early)

        # =========== protein attention partial (fills compound AG) =======
        scope("watt")
        S_pro_part = sb.tile([DIM, 1], f32, name="S_pro_part")
        attention_tail(hs_w, RW, RW, h_qp, S_pro_part, "pat")
        escope()

        # =========== compound conv + substrate attention =================
        # --- compound conv: center chunks read xsT directly (independent
        # of the halo AllGather); only 32-col edge chunks wait for it. ---
        scope("cconv")

        def cc_chunk(bufout, l, cs, n, rhs_fn):
            pscv = ps_mm.tile([DIM, 512], f32, name=f"ccv_ps{l}", tag="mm")
            for dri in range(11):
                nc.tensor.matmul(
                    out=pscv[0:DIM, 0:n], lhsT=convw_l(l, dri),
                    rhs=rhs_fn(cs + dri, n),
                    start=(dri == 0), stop=(dri == 10),
                )
            nc.scalar.activation(
                out=bufout[0:DIM, cs : cs + n], in_=pscv[0:DIM, 0:n],
                func=AF.Relu, bias=bcol(C_CONV + l),
            )

        spans = [RA + 21, RA + 11, RA + 1]  # 646, 636, 626
        cbufs = [
            sb.tile([DIM, spans[l]], f32r, name=f"ccv_b{l}", tag=f"ccv_b{l}")
            for l in range(3)
        ]
        # center chunks first: conv1 center reads xsT, later centers read the
        # previous buffer's center region only.
        for l in range(3):
            span = spans[l]
            ctr_lo, ctr_hi = 32, span - 32
            if l == 0:
                rf = lambda off, n: xsT[0:DIM, off - 15 : off - 15 + n]
            else:
                prev = cbufs[l - 1]
                rf = lambda off, n, prev=prev: prev[0:DIM, off : off + n]
            w = ctr_hi - ctr_lo
            for (cs, n) in _chunks2(w):
                cc_chunk(cbufs[l], l, ctr_lo + cs, n, rf)

        # halo extraction after AG
        xall = sb.tile([NC * DIM, 32], f32, name="xall")
        nc.sync.dma_start(out=xall[:, :], in_=cg_out[:, :])
        cbuf0 = sb.tile([DIM, RA + 31], f32r, name="cbuf0")
        for (side, sel, rcols, dcol) in (
            ("L", selL, (16, 32), 0),
            ("R", selR, (0, 16), RA + 15),
        ):
            ph = ps_tr.tile([DIM, 16], f32, name=f"chalo{side}", tag="tr")
            nc.tensor.matmul(
                out=ph[0:DIM, 0:16], lhsT=sel[0 : NC * DIM, 0:DIM],
                rhs=r32(xall[0 : NC * DIM, rcols[0] : rcols[1]]),
            )
            if side == "L":
                nc.vector.tensor_copy(cbuf0[0:DIM, 0:15], ph[0:DIM, 1:16])
            else:
                nc.vector.tensor_copy(
                    cbuf0[0:DIM, dcol : dcol + 15], ph[0:DIM, 0:15]
                )
        nc.vector.tensor_copy(cbuf0[0:DIM, 15:42], xsT[0:DIM, 0:27])
        nc.vector.tensor_copy(cbuf0[0:DIM, RA - 11 : RA + 15], xsT[0:DIM, RA - 26 : RA])
        nc.sync.dma_start(
            out=cbuf0[0:DIM, RA + 30 : RA + 31], in_=zeros_d[0:1, 0:DIM]
        )
        apply_edge_masks(cbuf0, RA + 30, maskc, 0)

        # edge chunks + masks per layer
        for l in range(3):
            span = spans[l]
            if l == 0:
                rf = lambda off, n: cbuf0[0:DIM, off : off + n]
            else:
                prev = cbufs[l - 1]
                rf = lambda off, n, prev=prev: prev[0:DIM, off : off + n]
            cc_chunk(cbufs[l], l, 0, 32, rf)
            cc_chunk(cbufs[l], l, span - 32, 32, rf)
            if l < 2:
                apply_edge_masks(cbufs[l], span - 1, maskc, 5 * (l + 1))

        hs_c = hs_from(cbufs[2], RA + 1, "hs_c", "cat")
        S_sub_part = sb.tile([DIM, 1], f32, name="S_sub_part")
        attention_tail(hs_c, RA + 1, RA, h_q, S_sub_part, "cat")
        escope()

        # =========== AllReduce of S_sub / S_pro ==========================
        scope("tail")
        ar2_in = dr.tile([2, DIM], f32, name="ar2_in")
        ar2_out = dr.tile([2, DIM], f32, name="ar2_out",
                          addr_space="Local" if stub_collectives else "Shared")
        nc.sync.dma_start(out=ar2_in[0:1, 0:DIM], in_=S_sub_part[0:DIM, 0:1])
        nc.sync.dma_start(out=ar2_in[1:2, 0:DIM], in_=S_pro_part[0:DIM, 0:1])
        if stub_collectives:
            nc.sync.dma_start(out=ar2_out[:, :], in_=ar2_in[:, :])
        else:
            nc.gpsimd.collective_compute(
                "AllReduce", ALU.add, replica_groups=RG,
                ins=[ar2_in[:, :].opt()], outs=[ar2_out[:, :].opt()],
            )
        S_sub = sb.tile([DIM, 1], f32, name="S_sub")
        S_pro = sb.tile([DIM, 1], f32, name="S_pro")
        nc.sync.dma_start(out=S_sub[0:DIM, 0:1], in_=ar2_out[0:1, 0:DIM])
        nc.sync.dma_start(out=S_pro[0:DIM, 0:1], in_=ar2_out[1:2, 0:DIM])

        # =========== pairwise + output MLP (replicated) ==================
        def lrelu_row(s_tile, wblk, name):
            # row = leaky_relu(s^T @ W + b) : [1, 10]
            p = ps_mm.tile([1, 512], f32, name=f"{name}_ps", tag="mm")
            nc.tensor.matmul(
                out=p[0:1, 0:DIM], lhsT=s_tile[0:DIM, 0:1], rhs=c32(s10w(wblk))
            )
            t0 = sb.tile([1, DIM], f32, name=f"{name}_t0")
            nc.vector.tensor_add(t0[0:1, 0:DIM], p[0:1, 0:DIM], wpbrow[0:1, 0:DIM])
            t1 = sb.tile([1, DIM], f32, name=f"{name}_t1")
            nc.vector.tensor_scalar_mul(t1[0:1, 0:DIM], t0[0:1, 0:DIM], 0.01)
            row = sb.tile([1, DIM], f32, name=name)
            nc.vector.tensor_tensor(
                out=row[0:1, 0:DIM], in0=t0[0:1, 0:DIM], in1=t1[0:1, 0:DIM],
                op=mybir.AluOpType.max,
            )
            return row

        pcf = lrelu_row(S_sub, B_WP_SUB, "pcf")
        ppf = lrelu_row(S_pro, B_WP_PRO, "ppf")
        # hid_T = sigmoid(pcf^T @ ppf)  [10, 10]
        phid = ps_mm.tile([DIM, 512], f32, name="phid", tag="mm")
        nc.tensor.matmul(
            out=phid[0:DIM, 0:DIM], lhsT=pcf[0:1, 0:DIM], rhs=ppf[0:1, 0:DIM]
        )
        hbuf = sb.tile([DIM, DIM], f32, name="hbuf0")
        nc.scalar.activation(
            out=hbuf[0:DIM, 0:DIM], in_=phid[0:DIM, 0:DIM], func=AF.Sigmoid
        )
        for l in range(L_NN):
            p = ps_mm.tile([DIM, 512], f32, name=f"po_ps{l}", tag="mm")
            nc.tensor.matmul(
                out=p[0:DIM, 0:DIM], lhsT=c32(s10w(B_PO + l)), rhs=hbuf[0:DIM, 0:DIM]
            )
            nbuf = sb2.tile([DIM, DIM], f32, name=f"hbuf{l + 1}", tag="hbuf")
            nc.scalar.activation(
                out=nbuf[0:DIM, 0:DIM], in_=p[0:DIM, 0:DIM],
                func=AF.Relu, bias=bcol(C_PO + l),
            )
            hbuf = nbuf
        # hid2 = relu(wp_plain.T @ hbuf + wp_b); S_pair = row sums
        S_pair = sb.tile([DIM, 1], f32, name="S_pair")
        p2 = ps_mm.tile([DIM, 512], f32, name="hid2_ps", tag="mm")
        nc.tensor.matmul(
            out=p2[0:DIM, 0:DIM], lhsT=c32(s10w(B_WP_PLAIN)), rhs=hbuf[0:DIM, 0:DIM]
        )
        h2_scr = sb.tile([DIM, DIM], f32, name="h2_scr")
        nc.scalar.activation(
            out=h2_scr[0:DIM, 0:DIM], in_=p2[0:DIM, 0:DIM], func=AF.Relu,
            bias=bcol(C_WP),
        )
        nc.vector.reduce_sum(
            out=S_pair[0:DIM, 0:1], in_=h2_scr[0:DIM, 0:DIM],
            axis=mybir.AxisListType.X,
        )
        # wo layer 1 splits the contraction: [S_sub; S_pro] via one rhs,
        # S_pair accumulated with a second matmul (no DRAM bounce).
        cat20 = sb.tile([2 * DIM, 1], f32, name="cat20")
        nc.sync.dma_start(out=cat20[0 : 2 * DIM, 0:1], in_=ar2_out[0:2, 0:DIM])
        p0 = ps_mm.tile([3 * DIM, 512], f32, name="wo_ps0", tag="mm")
        nc.tensor.matmul(
            out=p0[0 : 3 * DIM, 0:1], lhsT=w30[0 : 2 * DIM, 0:30],
            rhs=cat20[0 : 2 * DIM, 0:1], start=True, stop=False,
        )
        nc.tensor.matmul(
            out=p0[0 : 3 * DIM, 0:1], lhsT=wopair[0:DIM, 0:30],
            rhs=S_pair[0:DIM, 0:1], start=False, stop=True,
        )
        cat = sb2.tile([3 * DIM, 1], f32, name="cat1", tag="cat")
        nc.scalar.activation(
            out=cat[0 : 3 * DIM, 0:1], in_=p0[0 : 3 * DIM, 0:1],
            func=AF.Relu, bias=wob[0 : 3 * DIM, 0:1],
        )
        for l in range(1, L_OUT):
            p = ps_mm.tile([3 * DIM, 512], f32, name=f"wo_ps{l}", tag="mm")
            nc.tensor.matmul(
                out=p[0 : 3 * DIM, 0:1],
                lhsT=w30[0 : 3 * DIM, l * 30 : (l + 1) * 30],
                rhs=cat[0 : 3 * DIM, 0:1],
            )
            ncat = sb2.tile([3 * DIM, 1], f32, name=f"cat{l + 1}", tag="cat")
            nc.scalar.activation(
                out=ncat[0 : 3 * DIM, 0:1], in_=p[0 : 3 * DIM, 0:1],
                func=AF.Relu, bias=wob[0 : 3 * DIM, l : l + 1],
            )
            cat = ncat
        pf = ps_mm.tile([1, 512], f32, name="fin_ps", tag="mm")
        nc.tensor.matmul(
            out=pf[0:1, 0:1], lhsT=w30[0 : 3 * DIM, 90:91],
            rhs=cat[0 : 3 * DIM, 0:1],
        )
        res = sb.tile([1, 1], f32, name="res_sb")
        nc.scalar.activation(
            out=res[0:1, 0:1], in_=pf[0:1, 0:1], func=AF.Identity,
            bias=wib[0:1, 0:1],
        )
        nc.sync.dma_start(out=out_d[0:1, 0:1], in_=res[0:1, 0:1])
        escope()

    nc.compile()
    return nc


# ======================= host-side input prep ==========================

def _prep_weights(inp):
    Wc_k = np.asarray(inp["Wc_k"], np.float32).reshape(L_CNN, 11, 11)
    Wc_b = np.asarray(inp["Wc_b"], np.float32)
    convw = np.zeros((DIM, 330), np.float32)
    for l in range(L_CNN):
        for dri in range(11):
            dr = dri - WIN
            o = (l * 11 + dri) * 10
            for cin in range(DIM):
                for cout in range(DIM):
                    dc = cin - cout
                    if -WIN <= dc <= WIN:
                        convw[cin, o + cout] = Wc_k[l, dr + WIN, dc + WIN]

    blocks = []
    for i in range(L_GNN):
        blocks.append(inp["Wg_w"][i].T)
    for i in range(L_NN):
        blocks.append(inp["Wn_w"][i].T)
    for i in range(L_NN):
        blocks.append(inp["Po_w"][i].T)
    blocks.append(inp["Wa_w"].T)
    blocks.append(inp["Wa_w"].T / 200.0)
    blocks.append(inp["Wa_w"].T / L_PSSM)
    blocks.append(inp["Wp_w"].T / N_ATOMS)
    blocks.append(inp["Wp_w"].T / N_SEQ)
    blocks.append(inp["Wp_w"].T)
    small10 = np.concatenate(blocks, axis=1).astype(np.float32)  # [10, 150]

    b10 = np.zeros((DIM, 14), np.float32)
    for i in range(3):
        b10[:, C_GNN + i] = inp["Wg_b"][i]
        b10[:, C_WN + i] = inp["Wn_b"][i]
        b10[:, C_PO + i] = inp["Po_b"][i]
        b10[:, C_CONV + i] = Wc_b[i]
    b10[:, C_WA] = inp["Wa_b"]
    b10[:, C_WP] = inp["Wp_b"]

    wo0 = inp["Wo_w"][0].T.copy()
    scale = np.concatenate(
        [np.full(10, 1.0 / N_ATOMS), np.full(10, 1.0 / N_SEQ), np.full(10, 1.0 / DIM)]
    ).astype(np.float32)
    wo0 = wo0 * scale[:, None]
    w30 = np.concatenate(
        [wo0, inp["Wo_w"][1].T, inp["Wo_w"][2].T, inp["Wi_w"].T], axis=1
    ).astype(np.float32)  # [30, 91]
    wob = np.stack([inp["Wo_b"][l] for l in range(3)], axis=1).astype(np.float32)
    wo_pair = np.ascontiguousarray(w30[2 * DIM : 3 * DIM, 0:30])
    wpb_row = inp["Wp_b"].reshape(1, DIM).astype(np.float32)
    wib = inp["Wi_b"].reshape(1, 1).astype(np.float32)
    return convw, small10, b10, w30, wob, wpb_row, wib, wo_pair


def _swizzle_idx(idx, nch):
    pad = np.zeros(nch * 128, np.int32)
    pad[: len(idx)] = idx
    return np.ascontiguousarray(pad.reshape(nch, 128).T)  # [128, nch]


def _mask(a, span, total):
    raw = np.arange(a, a + span)
    return ((raw >= 0) & (raw < total)).astype(np.float32)[None].repeat(DIM, 0)


def _prep_in_maps(inputs):
    adj = np.ascontiguousarray(np.asarray(inputs["adjacency"], np.float32))
    emb_fp = np.ascontiguousarray(np.asarray(inputs["emb_fp"], np.float32))
    emb_word = np.ascontiguousarray(np.asarray(inputs["emb_word"], np.float32))
    fingerprints = np.asarray(inputs["fingerprints"]).astype(np.int32)
    words = np.asarray(inputs["words"]).astype(np.int32)
    pssms = np.asarray(inputs["pssms"], np.float32)
    rdkit = np.asarray(inputs["rdkitfeatures"], np.float32)

    wdict = {k: np.asarray(v, np.float32) for k, v in inputs.items()
             if k not in ("fingerprints", "words", "adjacency")}
    convw, small10, b10, w30, wob, wpb_row, wib, wo_pair = _prep_weights(wdict)

    rdkit_t = np.ascontiguousarray(rdkit.T)  # [10, 200]
    pssms_T = pssms.T  # [10, 2000]

    in_maps = []
    for c in range(NC):
        import ml_dtypes
        at = np.zeros((N_ATOMS, RA + 1), ml_dtypes.bfloat16)
        at[:, :RA] = adj[c * RA : (c + 1) * RA, :].T.astype(ml_dtypes.bfloat16)

        fp_idx = _swizzle_idx(fingerprints[c * RA : (c + 1) * RA], FP_CH)

        a_w = c * RW - HALO
        raw_w = np.arange(a_w, a_w + RW + 30)
        widx = _swizzle_idx(words[np.clip(raw_w, 0, N_SEQ - 1)], W_CH)

        a_c = c * RA - HALO
        selL = np.zeros((NC * DIM, DIM), np.float32)
        selR = np.zeros((NC * DIM, DIM), np.float32)
        if c > 0:
            for d in range(DIM):
                selL[(c - 1) * DIM + d, d] = 1.0
        if c < NC - 1:
            for d in range(DIM):
                selR[(c + 1) * DIM + d, d] = 1.0

        a_p = c * RP - HALO
        raw_p = np.arange(a_p, a_p + RP + 30)
        pb0 = np.zeros((DIM, RP + 36), np.float32)
        valid = (raw_p >= 0) & (raw_p < L_PSSM)
        pb0[:, : RP + 30][:, valid] = pssms_T[:, raw_p[valid]]

        in_maps.append(
            {
                "at_shard": at,
                "emb_fp": emb_fp,
                "emb_word": emb_word,
                "fp_idx": fp_idx,
                "word_idx": widx,
                "selL": selL,
                "selR": selR,
                "pssm_b0": pb0,
                "rdkit_t": rdkit_t,
                "mask_w": _mask(a_w, RW + 30, N_SEQ),
                "mask_c": _mask(a_c, RA + 30, N_ATOMS),
                "mask_p": _mask(a_p, RP + 30, L_PSSM),
                "convw": convw,
                "small10": small10,
                "b10": b10,
                "w30": w30,
                "wob": wob,
                "wo_pair": wo_pair,
                "wpb_row": wpb_row,
                "wib": wib,
                "ones10": np.ones((1, DIM), np.float32),
                "zeros10": np.zeros((1, DIM), np.float32),
            }
        )
    return in_maps


def kernel(**inputs):
    if "nc" not in _CACHE:
        _CACHE["nc"] = _build_program()
    nc = _CACHE["nc"]
    in_maps = _prep_in_maps(inputs)

    from concourse.bass_utils import run_bass_kernel_spmd

    kw = {}
    td = os.environ.get("BASS_KERNEL_TMPDIR")
    if td:
        kw["tmpdir"] = td
        kw["trace"] = True
    res = run_bass_kernel_spmd(nc, in_maps, core_ids=list(range(NC)), **kw)
    if res.exec_time_ns is not None:
        print(f"HW exec time: {res.exec_time_ns} ns")
    return np.asarray(res.results[0]["out"], np.float32)

